# revision 47
# baseline (speedup 1.0000x reference)
"""Distributed brute-force KNN (retrieval) kernel for 8 Trainium2 NeuronCores.

Strategy
--------
Candidates are sharded row-wise across the 8 cores (125k each). Each core
computes quantized scores for all 512 queries against its shard with the
tensor engine (bf16, exact integer-grid arithmetic). A per-column index
embedding u = iw * 2^-18 (iw = position within the 2048-wide window) is folded
into two augmented contraction rows; because all quantities are exact
multiples of 2^-18 below 2^6, the fp32 PSUM value carries BOTH the score and
the 11 index bits exactly.

The PSUM drain (the throughput bound: ~1.19 ns per score per engine, two
engines) is split per [128,1024] half-window between the two PSUM-capable
engines, load-balanced by their exact per-op costs:

  * DVE max8: top-8 packed (value+index) scores per half-window.
  * ACT relu+accum, in-place PSUM output (avoids the SBUF access penalty):
    the accumulated value IS the packed survivor when the half-window held
    exactly one; exact fp32 verification on the host rejects every other
    case, and those blocks are recovered by an exact host rescan.

Per-query thresholds ride in two augmented contraction rows (the matmul
directly produces s~ - t_q, keeping survivor values exactly representable
on the 2^-18 grid), as in the original scheme.

Host side picks thresholds from a random sample so ~50 candidates per core
per query survive (superset of the global top-k), decodes the embedded
indices, rescores survivors exactly, and merges to the global top-k.
"""

import numpy as np
import ml_dtypes

B, D, N = 512, 64, 1_000_000
NCORES = 8
NSHARD = N // NCORES            # 125000
WIN = 2048                      # max u-embedding window (11 index bits)
NPAD = 124928                   # device candidate count per core
NTAIL = NSHARD - NPAD           # 72 remainder candidates scanned on host
# Non-uniform zero-padding window plan: a small first window primes the
# DMA/compute pipeline fast; the rest are full 2048-wide windows.
WPLAN = [(0, 512)] \
    + [(512 + 2048 * k, 2048) for k in range(60)] \
    + [(512 + 2048 * 60, 1536)]


def _halves(wlen):
    if wlen == 512:
        return [(0, 512)]
    if wlen == 2048:
        return [(0, 1024), (1024, 1024)]
    return [(0, 1024), (1024, 512)]
KAUG = 68                       # 64 dims + t_hi + t_lo + u_hi + u_lo
QB = B // 128                   # 4 query blocks

SAMP = 32768                    # host-side sample size per core
RSTAR = 14                      # threshold = (RSTAR-th largest sample) - 1/128

_Q_GRID = 8.0                   # queries quantized to 1/8
_C_GRID = 16.0                  # candidates quantized to 1/16 -> score grid 1/128
_EMB = 2.0 ** 18                # index embedding unit 2^-18

# Per-unit engine costs (ns) used for load balancing: DVE max8 from PSUM is
# (fd+120)/0.96; ACT in-place relu+accum is fd/1.2 + 172/1.2 + 187 (accum
# readout). Both validated against the hardware cost model.
_COST_D = {1024: 1192.0, 512: 658.0}
_COST_A = {1024: 1184.0, 512: 757.0}


def _build_units():
    """PE-production-ordered (w, qb, hoff, fd) units + greedy engine
    assignment + per-qb output slot maps. Pure function of the constants."""
    units = []
    for w, (_ws, wlen) in enumerate(WPLAN):
        for qb in range(QB):
            for hoff, fd in _halves(wlen):
                units.append((w, qb, hoff, fd))
    d_load = a_load = 0.0
    eng = []
    for (_w, _qb, _h, fd) in units:
        cd, ca = _COST_D[fd], _COST_A[fd]
        if d_load + cd <= a_load + ca:
            eng.append("D")
            d_load += cd
        else:
            eng.append("A")
            a_load += ca
    dslot = [0] * QB
    aslot = [0] * QB
    slots = []
    for (w, qb, h, fd), e in zip(units, eng):
        if e == "D":
            slots.append(dslot[qb])
            dslot[qb] += 1
        else:
            slots.append(aslot[qb])
            aslot[qb] += 1
    return units, eng, slots, max(dslot), max(aslot)


UNITS, ENG, SLOTS, ND_MAX, NA_MAX = _build_units()

_CACHE = {}


def _build_bass():
    import concourse.tile as tile
    import concourse.mybir as mybir
    from concourse import bacc

    nc = bacc.Bacc("TRN2", target_bir_lowering=False, debug=False,
                   enable_asserts=False)
    q_dram = nc.dram_tensor("qaug", (KAUG, B), mybir.dt.bfloat16,
                            kind="ExternalInput")
    c_dram = nc.dram_tensor("caug", (KAUG, NPAD), mybir.dt.bfloat16,
                            kind="ExternalInput")
    out_dram = nc.dram_tensor("out_vals", (B, ND_MAX * 8), mybir.dt.float32,
                              kind="ExternalOutput")
    acc_dram = nc.dram_tensor("out_acc", (B, NA_MAX), mybir.dt.float32,
                              kind="ExternalOutput")

    with tile.TileContext(nc) as tc:
        with tc.tile_pool(name="cts", bufs=6) as ctp, \
             tc.tile_pool(name="qp", bufs=1) as qp, \
             tc.tile_pool(name="outp", bufs=1) as outp, \
             tc.tile_pool(name="ps", bufs=1, space="PSUM") as psp:

            qt = qp.tile([KAUG, B], mybir.dt.bfloat16)
            nc.sync.dma_start(qt[:], q_dram.ap()[:, :])

            ov = [outp.tile([128, ND_MAX * 8], mybir.dt.float32,
                            tag=f"ov{qb}", name=f"ov{qb}")
                  for qb in range(QB)]
            oa = [outp.tile([128, NA_MAX], mybir.dt.float32,
                            tag=f"oa{qb}", name=f"oa{qb}")
                  for qb in range(QB)]

            dcnt = acnt = 0
            ui = 0
            # output-DMA waves: flush completed ov/oa column ranges early so
            # the end-of-kernel tail only covers the last chunk
            done_d = [0] * QB
            done_a = [0] * QB
            dslot_now = [0] * QB
            aslot_now = [0] * QB
            WAVES = {15, 30, 45, 55, 59}
            last_wi = len(WPLAN) - 1
            for wi, (ws, wlen) in enumerate(WPLAN):
                ct = ctp.tile([KAUG, wlen], mybir.dt.bfloat16, tag="ct")
                nc.sync.dma_start(ct[:], c_dram.ap()[:, ws:ws + wlen])
                for qb in range(QB):
                    for hoff, fd in _halves(wlen):
                        (uw, uqb, uhoff, ufd) = UNITS[ui]
                        e, slot = ENG[ui], SLOTS[ui]
                        ui += 1
                        if e == "D":
                            tag = f"psD{dcnt % 2}"
                            dcnt += 1
                        else:
                            tag = f"psA{acnt % 2}"
                            acnt += 1
                        pt = psp.tile([128, 1024], mybir.dt.float32,
                                      tag=tag, name="pt")
                        for s in range(fd // 512):
                            col = hoff + s * 512
                            nc.tensor.matmul(pt[:, s * 512:(s + 1) * 512],
                                             qt[:, qb * 128:(qb + 1) * 128],
                                             ct[:, col:col + 512],
                                             start=True, stop=True)
                        if e == "D":
                            nc.vector.max(ov[qb][:, 8 * slot:8 * slot + 8],
                                          pt[:, :fd])
                            dslot_now[qb] = slot + 1
                        else:
                            nc.scalar.activation(
                                pt[:, :fd], pt[:, :fd],
                                mybir.ActivationFunctionType.Relu,
                                accum_out=oa[qb][:, slot:slot + 1])
                            aslot_now[qb] = slot + 1
                    if wi == last_wi:
                        # final flush per qb, each on its own queue so the
                        # tail DMAs don't serialize on one sequencer
                        eng_q = [nc.sync, nc.sync, nc.gpsimd,
                                 nc.scalar][qb]
                        r = slice(qb * 128, (qb + 1) * 128)
                        if ND_MAX > done_d[qb]:
                            c0 = 8 * done_d[qb]
                            eng_q.dma_start(out_dram.ap()[r, c0:],
                                            ov[qb][:, c0:])
                            done_d[qb] = ND_MAX
                        if NA_MAX > done_a[qb]:
                            c0 = done_a[qb]
                            eng_q.dma_start(acc_dram.ap()[r, c0:],
                                            oa[qb][:, c0:])
                            done_a[qb] = NA_MAX
                if wi in WAVES:
                    # waves go out on the SWDGE (Pool) queue: SP.SEQ must stay
                    # free for the candidate-window DMA stream
                    for qb in range(QB):
                        r = slice(qb * 128, (qb + 1) * 128)
                        if dslot_now[qb] > done_d[qb]:
                            c0, c1 = 8 * done_d[qb], 8 * dslot_now[qb]
                            nc.gpsimd.dma_start(out_dram.ap()[r, c0:c1],
                                                ov[qb][:, c0:c1])
                            done_d[qb] = dslot_now[qb]
                        if aslot_now[qb] > done_a[qb]:
                            c0, c1 = done_a[qb], aslot_now[qb]
                            nc.gpsimd.dma_start(acc_dram.ap()[r, c0:c1],
                                                oa[qb][:, c0:c1])
                            done_a[qb] = aslot_now[qb]

            for qb in range(QB):
                # safety net: flush anything not already covered above
                r = slice(qb * 128, (qb + 1) * 128)
                if ND_MAX > done_d[qb]:
                    c0 = 8 * done_d[qb]
                    nc.sync.dma_start(out_dram.ap()[r, c0:],
                                      ov[qb][:, c0:])
                if NA_MAX > done_a[qb]:
                    c0 = done_a[qb]
                    nc.sync.dma_start(acc_dram.ap()[r, c0:],
                                      oa[qb][:, c0:])
    nc.compile()
    return nc


def _get_nc():
    if "nc" not in _CACHE:
        _CACHE["nc"] = _build_bass()
    return _CACHE["nc"]


def _bf16(a):
    """Exact fp32->bf16 for values already representable in bf16 (bit shift;
    truncation == rounding here)."""
    return (np.ascontiguousarray(a, np.float32).view(np.uint32) >> 16) \
        .astype(np.uint16).view(ml_dtypes.bfloat16)


def _prep_inputs(queries, candidates):
    """Host-side staging: quantize, sample thresholds, build augmented operands."""
    qq = np.round(queries.astype(np.float32) * _Q_GRID) / _Q_GRID
    cc = np.round(candidates.astype(np.float32) * _C_GRID) / _C_GRID

    rng = np.random.default_rng(0x5EED)
    iw = np.empty(NPAD, dtype=np.int64)
    for ws, wlen in WPLAN:
        iw[ws:ws + wlen] = np.arange(wlen)
    u_hi = ((iw >> 6).astype(np.float32)) * (2.0 ** -12)   # 5 bits, bf16-exact
    u_lo = ((iw & 63).astype(np.float32)) * (2.0 ** -18)   # 6 bits, bf16-exact

    in_maps = []
    t_all = np.zeros((NCORES, B), np.float32)
    for c in range(NCORES):
        shard = cc[c * NSHARD:(c + 1) * NSHARD]            # [125000, 64]
        sidx = rng.choice(NSHARD, SAMP, replace=False)
        s_samp = qq @ shard[sidx].T                        # [512, SAMP] exact fp32
        t_raw = np.partition(s_samp, SAMP - RSTAR, axis=1)[:, SAMP - RSTAR]
        t = (t_raw - np.float32(1.0 / 128.0)).astype(np.float32)
        t_all[c] = t                                       # on grid, strictly below
        t_hi = np.floor(t)
        t_lo = (t - t_hi).astype(np.float32)

        qaug = np.zeros((KAUG, B), np.float32)
        qaug[:D] = qq.T
        qaug[D] = -t_hi
        qaug[D + 1] = -t_lo
        qaug[D + 2] = 1.0
        qaug[D + 3] = 1.0

        caug = np.zeros((KAUG, NPAD), np.float32)
        caug[:D] = shard[:NPAD].T
        caug[D] = 1.0
        caug[D + 1] = 1.0
        caug[D + 2] = u_hi
        caug[D + 3] = u_lo

        in_maps.append({"qaug": _bf16(qaug), "caug": _bf16(caug)})
    return in_maps, qq, cc, t_all


def _u_of(iw):
    """Exact fp32 embedding offset u(iw), matching the device aug rows."""
    return (((iw >> 6).astype(np.float32) * np.float32(2.0 ** -12))
            + (iw & 63).astype(np.float32) * np.float32(2.0 ** -18))


def _decode_and_merge(queries, candidates, core_outs, qq, cc, t_all, k):
    """Decode embedded indices, rescore survivors exactly, global top-k.

    DVE units: top-8 packed values (s~ + u) per [*,fd] half-window; survivors
    are decoded and filtered against t on the host. ACT units: relu+accum of
    (s~ + u - t); the accumulated value IS the packed survivor when the block
    held exactly one; exact-fp32 verification rejects every other case and
    those blocks are recovered by an exact host rescan on the quantized grid.
    """
    qn, cidx_all = [], []
    rescan = []                                            # (core, q, col0, fd)
    for c, (ovs, oas) in enumerate(core_outs):
        ovs = np.asarray(ovs, np.float32)                  # [B, ND_MAX*8]
        oas = np.asarray(oas, np.float32)                  # [B, NA_MAX]
        # exact host scan of the NTAIL remainder candidates (grid fp32)
        lo = c * NSHARD + NPAD
        s_tail = qq @ cc[lo:lo + NTAIL].T                  # [B, NTAIL]
        r, cnd = np.nonzero(s_tail > t_all[c][:, None])
        qn.append(r)
        cidx_all.append(lo + cnd)
        # Collect per-unit info vectorized per engine via precomputed maps.
        for (w, qb, hoff, fd), e, slot in zip(UNITS, ENG, SLOTS):
            ws = WPLAN[w][0]
            col0 = ws + hoff
            rows = slice(qb * 128, (qb + 1) * 128)
            if e == "D":
                v = ovs[rows, 8 * slot:8 * slot + 8]       # [128, 8]
                qi, sl = np.nonzero(v > 0)
                m = np.rint(v[qi, sl].astype(np.float64) * _EMB).astype(np.int64)
                iw = m & (WIN - 1)
                good = (iw >= hoff) & (iw < hoff + fd)
                qi, iw = qi[good], iw[good]
                qn.append(qb * 128 + qi)
                cidx_all.append(ws + iw + c * NSHARD)
            else:
                a = oas[rows, slot]                        # [128]
                qi = np.nonzero(a > 0)[0]
                if qi.size == 0:
                    continue
                av = a[qi]
                m = np.rint(av.astype(np.float64) * _EMB).astype(np.int64)
                iw = m & (WIN - 1)
                cand_local = ws + iw
                inb = (iw >= hoff) & (iw < hoff + fd)
                vc = np.full(av.shape, np.float32(np.nan), np.float32)
                if inb.any():
                    s_ex = np.einsum("md,md->m",
                                     qq[qb * 128 + qi[inb]],
                                     cc[c * NSHARD + cand_local[inb]],
                                     dtype=np.float32, casting="no")
                    vc[inb] = (s_ex - t_all[c, qb * 128 + qi[inb]]) \
                        .astype(np.float32) \
                        + _u_of(iw[inb]).astype(np.float32)
                good = inb & (vc == av)
                qn.append(qb * 128 + qi[good])
                cidx_all.append(cand_local[good] + c * NSHARD)
                for q in qi[~good]:
                    rescan.append((c, qb * 128 + q, col0, fd))
    # --- rescan unresolved ACT blocks with exact grid arithmetic ---
    if rescan:
        from collections import defaultdict
        groups = defaultdict(list)
        for c, q, col0, fd in rescan:
            groups[(c, col0, fd)].append(q)
        for (c, col0, fd), qs in groups.items():
            qs = np.array(qs)
            lo = c * NSHARD + col0
            hi = min(lo + fd, (c + 1) * NSHARD)
            if hi <= lo:
                continue
            s_blk = qq[qs] @ cc[lo:hi].T                   # exact fp32 grid
            r, cnd = np.nonzero(s_blk > t_all[c, qs][:, None])
            qn.append(qs[r])
            cidx_all.append(lo + cnd)
    qi = np.concatenate(qn)
    ci = np.concatenate(cidx_all)

    # exact rescore of survivors in float64, then order like jax.lax.top_k
    qf = queries.astype(np.float64)
    cf = candidates.astype(np.float64)
    vals = np.einsum("md,md->m", qf[qi], cf[ci])
    vals32 = vals.astype(np.float32)

    order = np.lexsort((ci, -vals, qi))
    qi, ci, vals32 = qi[order], ci[order], vals32[order]
    counts = np.bincount(qi, minlength=B)

    out_v = np.zeros((B, k), np.float32)
    out_i = np.zeros((B, k), np.int32)
    starts = np.concatenate(([0], np.cumsum(counts)))
    for b in range(B):
        s, e = starts[b], starts[b + 1]
        if e - s < k:   # statistical fallback — should essentially never happen
            sc = queries[b].astype(np.float64) @ candidates.astype(np.float64).T
            top = np.argpartition(-sc, k)[:k]
            top = top[np.lexsort((top, -sc[top]))]
            out_v[b] = sc[top].astype(np.float32)
            out_i[b] = top.astype(np.int32)
            continue
        out_v[b] = vals32[s:s + k]
        out_i[b] = ci[s:s + k].astype(np.int32)
    return out_v, out_i


def kernel(queries, candidates, k):
    import os
    from concourse import bass_utils

    k = int(k)
    queries = np.asarray(queries, np.float32)
    candidates = np.asarray(candidates, np.float32)
    in_maps, qq, cc, t_all = _prep_inputs(queries, candidates)
    nc = _get_nc()
    trace = os.environ.get("KNN_TRACE", "0") == "1"
    try:
        res = bass_utils.run_bass_kernel_spmd(nc, in_maps,
                                              core_ids=list(range(NCORES)),
                                              trace=trace)
    except ModuleNotFoundError:
        res = bass_utils.run_bass_kernel_spmd(nc, in_maps,
                                              core_ids=list(range(NCORES)))
    _CACHE["last_results"] = res
    core_outs = [(r["out_vals"], r["out_acc"]) for r in res.results]
    return _decode_and_merge(queries, candidates, core_outs, qq, cc, t_all, k)
